# revision 1
# baseline (speedup 1.0000x reference)
"""Trainium2 Bass kernel for nn_BertSelfAttention_79577154060613.

Block-sparse BERT self-attention (block-diagonal over 10 candidate blocks of
64 tokens + dense global columns for 128 term tokens), data-parallel over
batch across 8 NeuronCores (2 batches per core).

Key algorithmic trick: the reference multiplies scores by the mask (masked
entries become exactly 0, not -inf), so softmax gives each masked key weight
exp(0)=1. For a query in block c:
    ctx = (sum_{k in block c | terms} e^{s_k} v_k + sum_{c' != c} Vsum_c') / Z
    Z   = sum_{k in block c | terms} e^{s_k} + 9*64
where Vsum_c' are per-head, per-block sums of candidate value rows. This
turns 768-wide attention into 192-wide attention plus one small K=10 matmul
(lhsT = 1 - one_hot(c)) per query tile.

Performance structure (vs the bf16 baseline):
  - All projection matmuls (Q, K, V-candidates, Vsum) run in fp8 e4m3 with
    MatmulPerfMode.DoubleRow: two 128-row contraction planes per
    instruction at 0.5 cycles/output-column (4x fewer PE cycles than bf16).
    X and W are cast to fp8 on the host at natural scale; the fp8 rounding
    noise is diluted by softmax averaging (Z ~ 700) everywhere it matters.
  - The 128 term-token V rows pass straight to the output, so that slice
    (mt=5) uses a second-order residual-fp8 product (X8'W8 + (X/64)'(64dW)
    + dX8'W8), accurate to ~0.2% with fp8-only inputs.
  - Score matmuls also run fp8 DoubleRow: Q^T/K^T are stored as
    [32 partitions, 2 dh-planes, tokens] per head (W columns host-permuted
    so each projection out-tile is (4 heads x 32 dh-low | dh-high)), making
    the dh=64 contraction a 2-plane fp8 contraction. Because dual-fp8
    matmuls must write PSUM starting at partition 0, block-diagonal scores
    use a pair-quadrant form: lhsT spans TWO blocks (128 key rows), rhs is
    one block's 64 q columns, so each matmul yields [128, 64] of which one
    64-row half is wanted (the other half is junk that exp processes and
    PV never reads).
  - Per-block value sums come from host-precomputed per-block column sums
    of X (Xsum [H, 10], padded to 16 for dual-fp8 ldweights alignment)
    via one tiny fp8 matmul chain.
  - One head's scores live in a 2-bank psT tile (terms + blocks c8/c9,
    768 cols) and a 1-bank psU tile (blocks c0..c7, 512 cols), exp'd by
    two activation instructions. PSUM: psT x3 bufs (6 banks) + psU x2
    (shared with the PV accumulators) = 8 banks; the 3-deep psT rotation
    keeps exp/copy drains off the PE critical path.
  - The pair-quadrant junk is zeroed in se by two strided GPSIMD
    memsets, letting the PV correction and block matmuls run full-height:
    one 260-col correction matmul and one 65-col matmul per (head, pair)
    instead of two half-height ones - half the PE cost of the naive form.
  - Elementwise is balanced: exps + 2 K tiles + vterm -> ACT; Q, 4 K
    tiles, V, vsum, recip, divide-mult -> DVE; junk masks -> GPSIMD.
  - Outputs are staged into [128, H] bf16 SBUF tiles (host casts back to
    fp32), DMA'd in column-split halves as PV groups complete; the whole
    second batch's projection/score stream is woven between the first
    batch's PV chunks (software pipelining across batches).

PSUM discipline: start=True lazily zeroes the whole bank for the written
partitions; every dual-fp8 matmul writes full-height at partition 0, so
each bank's group is opened by its first full-height matmul and closed by
stop=True on its last.
"""

import numpy as np
import ml_dtypes

import concourse.bass as bass
import concourse.mybir as mybir
import concourse.tile as tile
from concourse import bacc
from concourse.bass_utils import run_bass_kernel_spmd

# Problem dims (hardcoded per contract)
B, CDD, L, T, H, NH = 16, 10, 64, 128, 768, 12
DH = H // NH  # 64
S = CDD * L + T  # 768
NQ = CDD * L  # 640
P = 128
NCORES = 8
BL = B // NCORES  # 2 batches per core
KT = H // P  # 6 contraction tiles
KTP = KT // 2  # 3 fp8 DoubleRow contraction pairs
CDDP = 16  # Xsum padded block count (dual-fp8 ldweights alignment)
FP32 = mybir.dt.float32
BF16 = mybir.dt.bfloat16
FP8 = mybir.dt.float8e4
AF = mybir.ActivationFunctionType
ALU = mybir.AluOpType
DR = mybir.MatmulPerfMode.DoubleRow
HGS = 4  # heads per attention group
NHG = NH // HGS  # 3 groups
VW = DH + 1  # value width per head incl. ones column (65)

# se layout (bf16, [128, 1280]): terms q0:512 | terms q512:640 | blocks
# c8,c9 | blocks c0..c7. The first 768 columns are exp'd from the 2-bank
# psT tile (terms + c8/c9), the rest from the 1-bank psU tile (c0..c7).
SW = 1280


def _tcol(j):
    # column of query chunk j*128 in se's terms region
    return j * P if j < 4 else 512


def _bcol(c):
    # column of block c's 64 q in se
    return 640 + (c - 8) * L if c >= 8 else 768 + c * L


def _pcol(j):
    # column of block PAIR j's 128 q in se (pairs are contiguous)
    return 768 + j * P if j < 4 else 640


def _build_program():
    nc = bacc.Bacc(
        "TRN2", target_bir_lowering=False, debug=False, num_devices=NCORES
    )
    x8 = nc.dram_tensor("x8", [BL, H, S], FP8, kind="ExternalInput").ap()
    dxt8 = nc.dram_tensor("dxt8", [BL, H, T], FP8, kind="ExternalInput").ap()
    xt64 = nc.dram_tensor("xt64", [BL, H, T], FP8, kind="ExternalInput").ap()
    xs8 = nc.dram_tensor("xs8", [BL, H, CDDP], FP8, kind="ExternalInput").ap()
    wq8 = nc.dram_tensor("wq8", [H, H], FP8, kind="ExternalInput").ap()
    wk8 = nc.dram_tensor("wk8", [H, H], FP8, kind="ExternalInput").ap()
    wv8 = nc.dram_tensor("wv8", [H, H], FP8, kind="ExternalInput").ap()
    dwv8 = nc.dram_tensor("dwv8", [H, H], FP8, kind="ExternalInput").ap()
    bq = nc.dram_tensor("bq", [H], FP32, kind="ExternalInput").ap()
    bk = nc.dram_tensor("bk", [H], FP32, kind="ExternalInput").ap()
    bv16 = nc.dram_tensor("bv16", [H], BF16, kind="ExternalInput").ap()
    out = nc.dram_tensor("out", [BL, S, H], BF16, kind="ExternalOutput").ap()

    with tile.TileContext(nc) as tc:
        _emit(tc, nc, x8, dxt8, xt64, xs8, wq8, wk8, wv8, dwv8, bq, bk, bv16, out)
    nc.compile()
    return nc


def _emit(tc, nc, x8, dxt8, xt64, xs8, wq8, wk8, wv8, dwv8, bq, bk, bv16, out):
    from contextlib import ExitStack

    ctx = ExitStack()
    with ctx:
        cpool = ctx.enter_context(tc.tile_pool(name="consts", bufs=1))
        wpool = ctx.enter_context(tc.tile_pool(name="weights", bufs=1))
        xtp = ctx.enter_context(tc.tile_pool(name="xt", bufs=2))
        qkv = ctx.enter_context(tc.tile_pool(name="qkv", bufs=2))
        sep = ctx.enter_context(tc.tile_pool(name="se", bufs=1))
        osp = ctx.enter_context(tc.tile_pool(name="osb", bufs=2))
        smp = ctx.enter_context(tc.tile_pool(name="small", bufs=2))
        psp = ctx.enter_context(tc.tile_pool(name="psum", bufs=1, space="PSUM"))

        def psbig():
            # 2-bank tile: projections ([*, 0:768]) and per-head terms+c8/c9
            # scores
            return psp.tile(
                [P, 768], FP32, tag="psT", bufs=3, name="psT",
                padded_shape=[P, 1024],
            )

        def psu():
            # 1-bank tile shared by block scores (c0..c7) and PV accumulators
            return psp.tile(
                [P, 512], FP32, tag="psU", bufs=2, name="psU",
            )

        # ---- constants ----
        onesrow = cpool.tile([1, P], BF16)  # 1.0 row (rank-1 lhsT)
        nc.gpsimd.memset(onesrow[:], 1.0)
        # tiny dummy exp so ACT's activation-table load (~1.3us) happens
        # during the initial DMA wait instead of before the first real exp
        dummy = cpool.tile([1, 1], FP32)
        nc.scalar.activation(dummy[:], onesrow[0:1, 0:1], AF.Exp, scale=0.125)
        zrow = cpool.tile([1, 1], BF16)  # 0.0 (group-closer rank-1 rhs)
        nc.gpsimd.memset(zrow[:], 0.0)
        # notselC[p, c*64+j] = 0 if p == c else 1  (p in 0..9)
        notselC = cpool.tile([CDD, NQ], BF16)
        nc.gpsimd.memset(notselC[:], 1.0)
        nc.gpsimd.affine_select(
            out=notselC.rearrange("p (c j) -> p c j", j=L),
            in_=notselC.rearrange("p (c j) -> p c j", j=L),
            compare_op=ALU.not_equal,
            fill=0.0,
            base=0,
            pattern=[[-1, CDD], [0, L]],
            channel_multiplier=1,
        )
        # Junk suppression for the pair-quadrant block scores: block c's
        # useful key rows are 64*(c%2); the other 64 rows are junk. A rank-2
        # fp8 DoubleRow matmul adds -96 to the junk quadrants IN PSUM, so
        # exp turns them into e^-12 ~ 0 and the block PV matmuls can run
        # full-height over a whole pair. jrow plane0 selects rows 0:64,
        # plane1 rows 64:128; jneg has -96 on the matching junk columns.

        # ---- weights & biases (shared by both batches) ----
        # x(b=0)/wq8 stream in ktpair-interleaved chunks so the Q projection
        # starts after ~1us and proceeds as pairs land; V weights (deferred)
        # arrive while Q/K compute.
        x_cur = {
            "x8": xtp.tile([P, KT, S], FP8, tag="x8", name="x8t"),
            "dxt8": xtp.tile([P, KT, T], FP8, tag="dxt8", name="dxt8t"),
            "xt64": xtp.tile([P, KT, T], FP8, tag="xt64", name="xt64t"),
            "xs8": xtp.tile([P, KT, CDDP], FP8, tag="xs8", name="xs8t"),
        }
        w_sb = {}
        for name, ap_ in (("q", wq8), ("k", wk8), ("v", wv8)):
            w_sb[name] = wpool.tile([P, KT, H], FP8, tag=f"w{name}8", name=f"w{name}8")
        dwv8_sb = wpool.tile([P, KT, H], FP8, tag="dwv8", name="dwv8sb")
        x8r = x8[0].rearrange("(kt p) s -> p kt s", p=P)
        wq8r = wq8.rearrange("(kt p) o -> p kt o", p=P)
        for kts in (slice(0, 2), slice(2, 4), slice(4, KT)):
            nc.sync.dma_start(out=x_cur["x8"][:, kts, :], in_=x8r[:, kts, :])
            nc.sync.dma_start(out=w_sb["q"][:, kts, :], in_=wq8r[:, kts, :])
        nc.sync.dma_start(
            out=w_sb["k"][:], in_=wk8.rearrange("(kt p) o -> p kt o", p=P)
        )
        b_col = {}
        for name, bap in (("q", bq), ("k", bk)):
            bc = cpool.tile([P, KT], FP32, tag=f"bc{name}", name=f"bcol{name}")
            nc.sync.dma_start(out=bc[:], in_=bap.rearrange("(t p) -> p t", p=P))
            b_col[name] = bc
        bv_row = cpool.tile([1, H], BF16)
        nc.sync.dma_start(out=bv_row[:], in_=bv16[None, :])
        nc.sync.dma_start(
            out=x_cur["xs8"][:], in_=xs8[0].rearrange("(kt p) s -> p kt s", p=P)
        )
        nc.sync.dma_start(
            out=w_sb["v"][:], in_=wv8.rearrange("(kt p) o -> p kt o", p=P)
        )
        nc.sync.dma_start(
            out=x_cur["dxt8"][:], in_=dxt8[0].rearrange("(kt p) s -> p kt s", p=P)
        )
        nc.sync.dma_start(
            out=x_cur["xt64"][:], in_=xt64[0].rearrange("(kt p) s -> p kt s", p=P)
        )
        nc.sync.dma_start(
            out=dwv8_sb[:], in_=dwv8.rearrange("(kt p) o -> p kt o", p=P)
        )
        bvb = cpool.tile([P, H], FP32)  # materialized [128, H] V bias

        # ---- per-batch x tiles (both batches up front; b>0 DMAs are
        # emitted mid-batch-0 as prefetch) ----
        xcs = [x_cur]
        for b in range(1, BL):
            xcs.append({
                "x8": xtp.tile([P, KT, S], FP8, tag="x8", name="x8t"),
                "dxt8": xtp.tile([P, KT, T], FP8, tag="dxt8", name="dxt8t"),
                "xt64": xtp.tile([P, KT, T], FP8, tag="xt64", name="xt64t"),
                "xs8": xtp.tile([P, KT, CDDP], FP8, tag="xs8", name="xs8t"),
            })

        def batch_units(b):
            """Return (pre, attn, tail) emission-thunk lists. Ordering is
            tuned for the 2-slot 3-bank PSUM rotation: consecutive psbig
            allocations alternate drain engines (ACT exps / DVE copies)
            with enough PE work in between to cover each drain."""
            xc = xcs[b]
            st = {}
            st["q8"] = [qkv.tile([P, 2, NQ], FP8, tag=f"q8{m}", name=f"q8{m}") for m in range(3)]
            st["k8"] = [qkv.tile([P, 2, S], FP8, tag=f"k8{m}", name=f"k8{m}") for m in range(3)]
            st["vext"] = [qkv.tile([P, NH * VW], BF16, tag=f"v{mt}", name=f"v{mt}") for mt in range(KT)]
            st["vterm"] = qkv.tile([P, H], BF16, tag="vterm", name="vterm")
            st["vsumsE"] = smp.tile([CDD, NH * VW], BF16, tag="vsums", name="vsumsE")
            st["se"] = [None] * NH
            st["ostage"] = [
                osp.tile([P, H], BF16, tag=f"os{j}", name=f"os{j}")
                for j in range(5)
            ]

            def proj_qk(name, t):
                def th():
                    dst, ntot = (st["q8"], NQ) if name == "q" else (st["k8"], S)
                    m, half = divmod(t, 2)
                    ts_ = slice(t * P, (t + 1) * P)
                    ps = psbig()
                    for n0 in range(0, ntot, 512):
                        nlen = min(512, ntot - n0)
                        for i in range(KTP):
                            nc.tensor.matmul(
                                ps[:, n0 : n0 + nlen],
                                lhsT=w_sb[name][:, 2 * i : 2 * i + 2, ts_],
                                rhs=xc["x8"][:, 2 * i : 2 * i + 2, n0 : n0 + nlen],
                                start=(i == 0),
                                stop=(i == KTP - 1),
                                perf_mode=DR,
                            )
                    if name == "k" and t < 2:
                        nc.scalar.activation(
                            dst[m][:, half, :],
                            ps[:, 0:ntot],
                            AF.Identity,
                            bias=b_col[name][:, t : t + 1],
                        )
                    else:
                        nc.vector.tensor_scalar_add(
                            dst[m][:, half, :],
                            ps[:, 0:ntot],
                            b_col[name][:, t : t + 1],
                        )
                return th

            def emit_scores(h):
                # terms + blocks c8/c9 in a 2-bank psT tile; blocks c0..c7
                # in a 1-bank psU tile. Pair-quadrant block matmuls (half
                # the rows junk) keep every dual-fp8 dst at partition 0.
                # Two exps per head, each draining its own tile.
                def th():
                    m, hl = divmod(h, HGS)
                    rows = slice(32 * hl, 32 * hl + 32)
                    tp = (32 * hl, 0)
                    q8, k8 = st["q8"], st["k8"]

                    def blockmm(ps, col, c, start, stop):
                        j = c // 2
                        js = slice(2 * j * L, (2 * j + 2) * L)  # block PAIR keys
                        cs = slice(c * L, (c + 1) * L)
                        nc.tensor.matmul(
                            ps[:, col : col + L],
                            lhsT=k8[m][rows, :, js],
                            rhs=q8[m][rows, :, cs],
                            start=start,
                            stop=stop,
                            perf_mode=DR,
                            skip_group_check=True,
                            tile_position=tp,
                        )

                    se = sep.tile([P, SW], BF16, tag=f"se{h}", name=f"se{h}")
                    pst = psbig()
                    nc.tensor.matmul(
                        pst[:, 0:512],
                        lhsT=k8[m][rows, :, NQ:S],
                        rhs=q8[m][rows, :, 0:512],
                        start=True,
                        stop=True,
                        perf_mode=DR,
                        tile_position=tp,
                    )
                    nc.tensor.matmul(
                        pst[:, 512:640],
                        lhsT=k8[m][rows, :, NQ:S],
                        rhs=q8[m][rows, :, 512:640],
                        start=True,
                        stop=False,
                        perf_mode=DR,
                        skip_group_check=True,
                        tile_position=tp,
                    )
                    blockmm(pst, 640, 8, False, False)
                    blockmm(pst, 704, 9, False, True)
                    nc.scalar.activation(
                        se[:, 0:768], pst[:, 0:768], AF.Exp, scale=0.125
                    )
                    psb = psu()
                    for c in range(8):
                        blockmm(psb, c * L, c, c == 0, c == 7)
                    nc.scalar.activation(
                        se[:, 768:SW], psb[:, 0:512], AF.Exp, scale=0.125
                    )
                    # zero the junk quadrants so block PV matmuls can run
                    # full-height: in se's block region [c8 c9 c0..c7], even
                    # region-index blocks have junk rows 64:128, odd 0:64
                    sbv = se[:, 640:SW].rearrange("p (c j) -> p c j", j=L)
                    nc.gpsimd.memset(sbv[64:128, 0::2, :], 0.0)
                    nc.gpsimd.memset(sbv[0:64, 1::2, :], 0.0)
                    st["se"][h] = se
                return th

            def v_copy(mt, ps):
                vv = st["vext"][mt].rearrange("p (h c) -> p h c", c=VW)
                nc.vector.tensor_tensor(
                    out=vv[:, :, 0:DH],
                    in0=ps[:, 0:H].rearrange("p (h c) -> p h c", c=DH),
                    in1=bvb[:].rearrange("p (h c) -> p h c", c=DH),
                    op=ALU.add,
                )
                nc.gpsimd.memset(vv[:, :, DH : DH + 1], 1.0)

            def make_bvb():
                # materialized V bias [128, H] fp32 (b==0 only)
                ps = psbig()
                for n0, nlen in ((0, 512), (512, 256)):
                    nc.tensor.matmul(
                        ps[:, n0 : n0 + nlen],
                        lhsT=onesrow[:],
                        rhs=bv_row[0:1, n0 : n0 + nlen],
                        start=True,
                        stop=True,
                    )
                nc.vector.tensor_copy(bvb[:], ps[:, 0:H])

            def v_mt5():
                # term rows feed the output directly, so they get a
                # second-order-accurate residual-fp8 product:
                #   X16'W16 ~ X8'W8 + X8'dW8 + dX8'W8   (X = X8 + dX, ...)
                # The bias comes in via a rank-1 PSUM opener so the two
                # drains (vterm on ACT, vext5 on DVE) run in parallel.
                ps = psbig()
                for n0, nlen in ((0, 512), (512, 256)):
                    nc.tensor.matmul(
                        ps[:, n0 : n0 + nlen],
                        lhsT=onesrow[:],
                        rhs=bv_row[0:1, n0 : n0 + nlen],
                        start=True,
                        stop=False,
                    )
                    for lhs, rhs in (
                        (xc["x8"], w_sb["v"]),
                        (xc["xt64"], dwv8_sb),
                        (xc["dxt8"], w_sb["v"]),
                    ):
                        for i in range(KTP):
                            lt = (
                                lhs[:, 2 * i : 2 * i + 2, NQ:S]
                                if lhs is xc["x8"]
                                else lhs[:, 2 * i : 2 * i + 2, :]
                            )
                            nc.tensor.matmul(
                                ps[:, n0 : n0 + nlen],
                                lhsT=lt,
                                rhs=rhs[:, 2 * i : 2 * i + 2, n0 : n0 + nlen],
                                start=False,
                                stop=(rhs is None),
                                perf_mode=DR,
                            )
                    # close the group with a rank-1 +0
                    nc.tensor.matmul(
                        ps[:, n0 : n0 + 1],
                        lhsT=onesrow[:],
                        rhs=zrow[:],
                        start=False,
                        stop=True,
                    )
                vv = st["vext"][5].rearrange("p (h c) -> p h c", c=VW)
                nc.vector.tensor_copy(
                    vv[:, :, 0:DH],
                    ps[:, 0:H].rearrange("p (h c) -> p h c", c=DH),
                )
                nc.gpsimd.memset(vv[:, :, DH : DH + 1], 1.0)
                nc.scalar.activation(st["vterm"][:], ps[:, 0:H], AF.Copy)
                # term rows pass through V - DMA out early
                nc.sync.dma_start(out=out[b][NQ:S, :], in_=st["vterm"][:])

            def v_mt(mt):
                # candidate rows: fp8 DoubleRow
                def th():
                    ms = slice(mt * P, (mt + 1) * P)
                    ps = psbig()
                    for n0, nlen in ((0, 512), (512, 256)):
                        for i in range(KTP):
                            nc.tensor.matmul(
                                ps[:, n0 : n0 + nlen],
                                lhsT=xc["x8"][:, 2 * i : 2 * i + 2, ms],
                                rhs=w_sb["v"][:, 2 * i : 2 * i + 2, n0 : n0 + nlen],
                                start=(i == 0),
                                stop=(i == KTP - 1),
                                perf_mode=DR,
                            )
                    v_copy(mt, ps)
                return th

            def vsum():
                # per-block value sums from host-precomputed Xsum:
                # Vsum_c = Xsum_c @ Wv + 64*bv; 65th col = 64.0 so the
                # notselC correction matmul also contributes 9*64 to Z.
                ps = psbig()
                for n0, nlen in ((0, 512), (512, 256)):
                    for i in range(KTP):
                        nc.tensor.matmul(
                            ps[0:CDD, n0 : n0 + nlen],
                            lhsT=xc["xs8"][:, 2 * i : 2 * i + 2, 0:CDD],
                            rhs=w_sb["v"][:, 2 * i : 2 * i + 2, n0 : n0 + nlen],
                            start=(i == 0),
                            stop=(i == KTP - 1),
                            perf_mode=DR,
                        )
                vsv = st["vsumsE"].rearrange("p (h c) -> p h c", c=VW)
                nc.vector.scalar_tensor_tensor(
                    out=vsv[:, :, 0:DH],
                    in0=bvb[0:CDD, :].rearrange("p (h c) -> p h c", c=DH),
                    scalar=float(L),
                    in1=ps[0:CDD, 0:H].rearrange("p (h c) -> p h c", c=DH),
                    op0=ALU.mult,
                    op1=ALU.add,
                )
                nc.gpsimd.memset(vsv[:, :, DH : DH + 1], float(L))

            def prefetch():
                if b + 1 < BL:
                    xn = xcs[b + 1]
                    nc.sync.dma_start(
                        out=xn["x8"][:],
                        in_=x8[b + 1].rearrange("(kt p) s -> p kt s", p=P),
                    )
                    nc.sync.dma_start(
                        out=xn["dxt8"][:],
                        in_=dxt8[b + 1].rearrange("(kt p) s -> p kt s", p=P),
                    )
                    nc.sync.dma_start(
                        out=xn["xt64"][:],
                        in_=xt64[b + 1].rearrange("(kt p) s -> p kt s", p=P),
                    )
                    nc.sync.dma_start(
                        out=xn["xs8"][:],
                        in_=xs8[b + 1].rearrange("(kt p) s -> p kt s", p=P),
                    )

            def emit_pv_j(hg, j):
                def th():
                    vext, vsumsE, se_h = st["vext"], st["vsumsE"], st["se"]
                    hgs_v = slice(hg * HGS * VW, (hg + 1) * HGS * VW)
                    psc = psu()
                    # head 0's full-height terms matmul opens the bank's one
                    # accumulation group; everything else accumulates.
                    for hl in range(HGS):
                        h = hg * HGS + hl
                        vs = slice(h * VW, (h + 1) * VW)
                        nc.tensor.matmul(
                            psc[:, hl * VW : (hl + 1) * VW],
                            lhsT=se_h[h][:, _tcol(j) : _tcol(j) + P],
                            rhs=vext[5][:, vs],
                            start=(hl == 0),
                            stop=False,
                        )
                    # both 64-row halves in one full-height matmul each: the
                    # pair's q columns are contiguous in notselC and se, and
                    # the junk quadrants of se are zeroed
                    nc.tensor.matmul(
                        psc[:, 0 : HGS * VW],
                        lhsT=notselC[:, 2 * j * L : (2 * j + 2) * L],
                        rhs=vsumsE[:, hgs_v],
                        start=False,
                        stop=False,
                    )
                    for hl in range(HGS):
                        h = hg * HGS + hl
                        vs = slice(h * VW, (h + 1) * VW)
                        nc.tensor.matmul(
                            psc[:, hl * VW : hl * VW + VW],
                            lhsT=se_h[h][:, _pcol(j) : _pcol(j) + P],
                            rhs=vext[j][:, vs],
                            start=False,
                            stop=(hl == HGS - 1),
                        )
                    zr = smp.tile([P, HGS], FP32, tag="zr", bufs=4, name="zr")
                    pscv = psc[:, 0 : HGS * VW].rearrange("p (h c) -> p h c", c=VW)
                    nc.vector.reciprocal(
                        zr[:].rearrange("p (h o) -> p h o", o=1),
                        pscv[:, :, DH : DH + 1],
                    )
                    ov = st["ostage"][j].rearrange("p (h c) -> p h c", c=DH)
                    bin0, bin1 = bass.broadcast_tensor_aps(
                        pscv[:, :, 0:DH],
                        zr[:].rearrange("p (h o) -> p h o", o=1),
                    )
                    nc.vector.tensor_tensor(
                        out=ov[:, hg * HGS : (hg + 1) * HGS, :],
                        in0=bin0,
                        in1=bin1,
                        op=ALU.mult,
                    )
                    # split the output DMA: first 2 groups' columns go out
                    # after PV(1,j); the last group's after PV(2,j)
                    if hg == 1:
                        nc.sync.dma_start(
                            out=out[b][j * P : (j + 1) * P, 0 : 2 * HGS * DH],
                            in_=st["ostage"][j][:, 0 : 2 * HGS * DH],
                        )
                    elif hg == 2:
                        nc.sync.dma_start(
                            out=out[b][j * P : (j + 1) * P, 2 * HGS * DH : H],
                            in_=st["ostage"][j][:, 2 * HGS * DH : H],
                        )
                return th

            def proj_q_triple(ts3):
                # pair-major over three Q tiles: all three advance as each
                # x8/wq8 ktpair chunk lands, so the DMA-gated startup window
                # keeps the PE fed instead of serializing per tile
                def th():
                    pss = [psbig() for _ in ts3]
                    for i in range(KTP):
                        for idx, t in enumerate(ts3):
                            ts_ = slice(t * P, (t + 1) * P)
                            for n0 in range(0, NQ, 512):
                                nlen = min(512, NQ - n0)
                                nc.tensor.matmul(
                                    pss[idx][:, n0 : n0 + nlen],
                                    lhsT=w_sb["q"][:, 2 * i : 2 * i + 2, ts_],
                                    rhs=xc["x8"][:, 2 * i : 2 * i + 2, n0 : n0 + nlen],
                                    start=(i == 0),
                                    stop=(i == KTP - 1),
                                    perf_mode=DR,
                                )
                    for idx, t in enumerate(ts3):
                        m, half = divmod(t, 2)
                        nc.vector.tensor_scalar_add(
                            st["q8"][m][:, half, :],
                            pss[idx][:, 0:NQ],
                            b_col["q"][:, t : t + 1],
                        )
                return th

            Q = [proj_qk("q", t) for t in range(KT)]
            K = [proj_qk("k", t) for t in range(KT)]
            SC = [emit_scores(h) for h in range(NH)]
            V = [v_mt(mt) for mt in range(5)]
            PV = {(hg, j): emit_pv_j(hg, j) for hg in range(NHG) for j in range(5)}
            pre = [
                Q[0], K[0], Q[1], K[1],
                SC[0], Q[2], K[2], SC[1], Q[3], K[3],
                SC[2], Q[4], K[4], SC[3], Q[5], K[5],
                prefetch,
            ]
            if b == 0:
                pre.append(make_bvb)
            pre += [
                vsum, SC[4], v_mt5, SC[5], V[0], SC[6], V[1], SC[7],
            ]
            # V[2..4] arrive just-in-time inside the attention weave
            attn = [
                SC[8], PV[0, 0], V[2], PV[0, 1],
                SC[9], PV[0, 2], V[3], PV[0, 3],
                SC[10], V[4], PV[0, 4], PV[1, 0],
                SC[11], PV[1, 1], PV[1, 2], PV[1, 3], PV[1, 4],
            ]
            tail = [PV[2, j] for j in range(5)]
            return pre, attn, tail

        u0 = batch_units(0)
        u1 = batch_units(1)
        for th in u0[0]:
            th()
        # weave batch 1's projections/scores into batch 0's attention and
        # final PV group: batch 0's PV chunks are pure PE work that covers
        # batch 1's ACT/DVE copy and exp drains (and vice versa)
        rest0 = u0[1] + u0[2]
        pre1 = u1[0]
        k = 0
        for i, th in enumerate(rest0):
            th()
            if i >= 8:  # let batch 0's attention pipeline fill first
                for _ in range(2):
                    if k < len(pre1):
                        pre1[k]()
                        k += 1
        while k < len(pre1):
            pre1[k]()
            k += 1
        for th in u1[1]:
            th()
        for th in u1[2]:
            th()


_CACHE = {}


def _get_program():
    if "nc" not in _CACHE:
        _CACHE["nc"] = _build_program()
    return _CACHE["nc"]


NPF8 = ml_dtypes.float8_e4m3
NPBF = ml_dtypes.bfloat16


def _make_in_maps(inputs):
    hs = np.asarray(inputs["hidden_states"], np.float32)
    hst = np.ascontiguousarray(hs.transpose(0, 2, 1))  # [B, H, S]
    x8 = hst.astype(NPF8)
    dxt8 = (
        hst[:, :, NQ:] - x8[:, :, NQ:].astype(np.float32)
    ).astype(NPF8)
    xt64 = (hst[:, :, NQ:] / 64.0).astype(NPF8)
    xsum = np.zeros((B, H, CDDP), np.float32)
    xsum[:, :, :CDD] = hst[:, :, :NQ].reshape(B, H, CDD, L).sum(axis=3)
    xs8 = xsum.astype(NPF8)

    # Q/K projection out-column permutation: out-tile t=2m+half holds
    # (head 4m + r//32, dh = 32*half + r%32) at partition r.
    r = np.arange(P)
    perm = np.empty(H, np.int64)
    for t in range(KT):
        m, half = divmod(t, 2)
        perm[t * P + r] = (HGS * m + r // 32) * DH + 32 * half + (r % 32)

    wq = np.asarray(inputs["Wq"], np.float32).T
    wk = np.asarray(inputs["Wk"], np.float32).T
    wv = np.asarray(inputs["Wv"], np.float32).T
    bqp = np.asarray(inputs["bq"], np.float32)[perm]
    bkp = np.asarray(inputs["bk"], np.float32)[perm]
    bv = np.asarray(inputs["bv"], np.float32)
    in_common = {
        "wq8": np.ascontiguousarray(wq[:, perm]).astype(NPF8),
        "wk8": np.ascontiguousarray(wk[:, perm]).astype(NPF8),
        "wv8": np.ascontiguousarray(wv).astype(NPF8),
        "dwv8": np.ascontiguousarray(
            64.0 * (wv - np.ascontiguousarray(wv).astype(NPF8).astype(np.float32))
        ).astype(NPF8),
        "bq": np.ascontiguousarray(bqp),
        "bk": np.ascontiguousarray(bkp),
        "bv16": bv.astype(NPBF),
    }
    return [
        {
            "x8": x8[i * BL : (i + 1) * BL],
            "dxt8": dxt8[i * BL : (i + 1) * BL],
            "xt64": xt64[i * BL : (i + 1) * BL],
            "xs8": xs8[i * BL : (i + 1) * BL],
            **in_common,
        }
        for i in range(NCORES)
    ]


def kernel(**inputs) -> np.ndarray:
    in_maps = _make_in_maps(inputs)
    nc = _get_program()
    res = run_bass_kernel_spmd(nc, in_maps, list(range(NCORES)))
    return np.concatenate(
        [res.results[i]["out"].astype(np.float32) for i in range(NCORES)], axis=0
    )



# revision 18
# speedup vs baseline: 1.0319x; 1.0319x over previous
"""Trainium2 Bass kernel for nn_BertSelfAttention_79577154060613.

Block-sparse BERT self-attention (block-diagonal over 10 candidate blocks of
64 tokens + dense global columns for 128 term tokens), data-parallel over
batch across 8 NeuronCores (2 batches per core).

Key algorithmic trick: the reference multiplies scores by the mask (masked
entries become exactly 0, not -inf), so softmax gives each masked key weight
exp(0)=1. For a query in block c:
    ctx = (sum_{k in block c | terms} e^{s_k} v_k + sum_{c' != c} Vsum_c') / Z
    Z   = sum_{k in block c | terms} e^{s_k} + 9*64
where Vsum_c' are per-head, per-block sums of candidate value rows. This
turns 768-wide attention into 192-wide attention plus one small K=10 matmul
(lhsT = 1 - one_hot(c)) per query tile.

Performance structure (vs the bf16 baseline):
  - All projection matmuls (Q, K, V-candidates, Vsum) run in fp8 e4m3 with
    MatmulPerfMode.DoubleRow: two 128-row contraction planes per
    instruction at 0.5 cycles/output-column (4x fewer PE cycles than bf16).
    X and W are cast to fp8 on the host at natural scale; the fp8 rounding
    noise is diluted by softmax averaging (Z ~ 700) everywhere it matters.
  - The 128 term-token V rows pass straight to the output, so that slice
    (mt=5) uses a second-order residual-fp8 product (X8'W8 + (X/64)'(64dW)
    + dX8'W8), accurate to ~0.2% with fp8-only inputs.
  - Score matmuls also run fp8 DoubleRow: Q^T/K^T are stored as
    [32 partitions, 2 dh-planes, tokens] per head (W columns host-permuted
    so each projection out-tile is (4 heads x 32 dh-low | dh-high)), making
    the dh=64 contraction a 2-plane fp8 contraction. Because dual-fp8
    matmuls must write PSUM starting at partition 0, block-diagonal scores
    use a pair-quadrant form: lhsT spans TWO blocks (128 key rows), rhs is
    one block's 64 q columns, so each matmul yields [128, 64] of which one
    64-row half is wanted (the other half is junk that exp processes and
    PV never reads).
  - Per-block value sums come from host-precomputed per-block column sums
    of X (Xsum [H, 10], padded to 16 for dual-fp8 ldweights alignment)
    via one tiny fp8 matmul chain.
  - One head's scores live in a 2-bank psT tile (terms + blocks c8/c9,
    768 cols) and a 1-bank psU tile (blocks c0..c7, 512 cols), exp'd by
    two activation instructions. PSUM: psT x3 bufs (6 banks) + psU x2
    (shared with the PV accumulators) = 8 banks; the 3-deep psT rotation
    keeps exp/copy drains off the PE critical path.
  - The pair-quadrant junk is zeroed in se by two strided GPSIMD
    memsets, letting the PV correction and block matmuls run full-height:
    one 260-col correction matmul and one 65-col matmul per (head, pair)
    instead of two half-height ones - half the PE cost of the naive form.
  - Elementwise is balanced: exps + 2 K tiles + vterm -> ACT; Q, 4 K
    tiles, V, vsum, recip, divide-mult -> DVE; junk masks -> GPSIMD.
  - Outputs are staged into [128, H] bf16 SBUF tiles (host casts back to
    fp32), DMA'd in column-split halves as PV groups complete; the whole
    second batch's projection/score stream is woven between the first
    batch's PV chunks (software pipelining across batches).

PSUM discipline: start=True lazily zeroes the whole bank for the written
partitions; every dual-fp8 matmul writes full-height at partition 0, so
each bank's group is opened by its first full-height matmul and closed by
stop=True on its last.
"""

import numpy as np
import ml_dtypes

import concourse.bass as bass
import concourse.mybir as mybir
import concourse.tile as tile
from concourse import bacc
from concourse.bass_utils import run_bass_kernel_spmd

# Problem dims (hardcoded per contract)
B, CDD, L, T, H, NH = 16, 10, 64, 128, 768, 12
DH = H // NH  # 64
S = CDD * L + T  # 768
NQ = CDD * L  # 640
P = 128
NCORES = 8
BL = B // NCORES  # 2 batches per core
KT = H // P  # 6 contraction tiles
KTP = KT // 2  # 3 fp8 DoubleRow contraction pairs
CDDP = 16  # Xsum padded block count (dual-fp8 ldweights alignment)
FP32 = mybir.dt.float32
BF16 = mybir.dt.bfloat16
FP8 = mybir.dt.float8e4
AF = mybir.ActivationFunctionType
ALU = mybir.AluOpType
DR = mybir.MatmulPerfMode.DoubleRow
HGS = 4  # heads per attention group
NHG = NH // HGS  # 3 groups
VW = DH + 1  # value width per head incl. ones column (65)

# se layout (bf16, [128, 1280]): terms q0:512 | terms q512:640 | blocks
# c8,c9 | blocks c0..c7. The first 768 columns are exp'd from the 2-bank
# psT tile (terms + c8/c9), the rest from the 1-bank psU tile (c0..c7).
SW = 1280


def _tcol(j):
    # column of query chunk j*128 in se's terms region
    return j * P if j < 4 else 512


def _bcol(c):
    # column of block c's 64 q in se
    return 640 + (c - 8) * L if c >= 8 else 768 + c * L


def _pcol(j):
    # column of block PAIR j's 128 q in se (pairs are contiguous)
    return 768 + j * P if j < 4 else 640


def _build_program():
    nc = bacc.Bacc(
        "TRN2", target_bir_lowering=False, debug=False, num_devices=NCORES
    )
    x8 = nc.dram_tensor("x8", [BL, H, S], FP8, kind="ExternalInput").ap()
    dxt8 = nc.dram_tensor("dxt8", [BL, H, T], FP8, kind="ExternalInput").ap()
    xt64 = nc.dram_tensor("xt64", [BL, H, T], FP8, kind="ExternalInput").ap()
    vse = nc.dram_tensor("vse", [BL, CDD, NH * VW], BF16, kind="ExternalInput").ap()
    wq8 = nc.dram_tensor("wq8", [H, H], FP8, kind="ExternalInput").ap()
    wk8 = nc.dram_tensor("wk8", [H, H], FP8, kind="ExternalInput").ap()
    wv8 = nc.dram_tensor("wv8", [H, H], FP8, kind="ExternalInput").ap()
    dwv8 = nc.dram_tensor("dwv8", [H, H], FP8, kind="ExternalInput").ap()
    bq = nc.dram_tensor("bq", [H], FP32, kind="ExternalInput").ap()
    bk = nc.dram_tensor("bk", [H], FP32, kind="ExternalInput").ap()
    bv16 = nc.dram_tensor("bv16", [H], BF16, kind="ExternalInput").ap()
    out = nc.dram_tensor("out", [BL, S, H], BF16, kind="ExternalOutput").ap()

    with tile.TileContext(nc) as tc:
        _emit(tc, nc, x8, dxt8, xt64, vse, wq8, wk8, wv8, dwv8, bq, bk, bv16, out)
    nc.compile()
    return nc


def _emit(tc, nc, x8, dxt8, xt64, vse, wq8, wk8, wv8, dwv8, bq, bk, bv16, out):
    from contextlib import ExitStack

    ctx = ExitStack()
    with ctx:
        cpool = ctx.enter_context(tc.tile_pool(name="consts", bufs=1))
        wpool = ctx.enter_context(tc.tile_pool(name="weights", bufs=1))
        xtp = ctx.enter_context(tc.tile_pool(name="xt", bufs=2))
        qkv = ctx.enter_context(tc.tile_pool(name="qkv", bufs=2))
        sep = ctx.enter_context(tc.tile_pool(name="se", bufs=1))
        osp = ctx.enter_context(tc.tile_pool(name="osb", bufs=2))
        smp = ctx.enter_context(tc.tile_pool(name="small", bufs=2))
        psp = ctx.enter_context(tc.tile_pool(name="psum", bufs=1, space="PSUM"))

        def psbig():
            # 2-bank tile: projections ([*, 0:768]) and per-head terms+c8/c9
            # scores
            return psp.tile(
                [P, 768], FP32, tag="psT", bufs=3, name="psT",
                padded_shape=[P, 1024],
            )

        def psu():
            # 1-bank tile shared by block scores (c0..c7) and PV accumulators
            return psp.tile(
                [P, 512], FP32, tag="psU", bufs=2, name="psU",
            )

        # ---- weight DMAs on the Pool queue, emitted FIRST so they
        # dispatch in parallel with the x8 stream on the SP queue ----
        w_sb = {}
        for name in ("q", "k", "v"):
            w_sb[name] = wpool.tile([P, KT, H], FP8, tag=f"w{name}8", name=f"w{name}8")
        dwv8_sb = wpool.tile([P, KT, H], FP8, tag="dwv8", name="dwv8sb")
        wq8r = wq8.rearrange("(kt p) o -> p kt o", p=P)
        for kts in (slice(0, 2), slice(2, 4), slice(4, KT)):
            nc.gpsimd.dma_start(out=w_sb["q"][:, kts, :], in_=wq8r[:, kts, :])
        nc.gpsimd.dma_start(
            out=w_sb["k"][:], in_=wk8.rearrange("(kt p) o -> p kt o", p=P)
        )
        nc.gpsimd.dma_start(
            out=w_sb["v"][:], in_=wv8.rearrange("(kt p) o -> p kt o", p=P)
        )
        nc.gpsimd.dma_start(
            out=dwv8_sb[:], in_=dwv8.rearrange("(kt p) o -> p kt o", p=P)
        )

        # ---- constants ----
        onesrow = cpool.tile([1, P], BF16)  # 1.0 row (rank-1 lhsT)
        nc.gpsimd.memset(onesrow[:], 1.0)
        # tiny dummy exp so ACT's activation-table load (~1.3us) happens
        # during the initial DMA wait instead of before the first real exp
        dummy = cpool.tile([1, 1], FP32)
        nc.scalar.activation(dummy[:], onesrow[0:1, 0:1], AF.Exp, scale=0.125)
        zrow = cpool.tile([1, 1], BF16)  # 0.0 (group-closer rank-1 rhs)
        nc.gpsimd.memset(zrow[:], 0.0)
        # notselC[p, c*64+j] = 0 if p == c else 1  (p in 0..9)
        notselC = cpool.tile([CDD, NQ], BF16)
        nc.gpsimd.memset(notselC[:], 1.0)
        nc.gpsimd.affine_select(
            out=notselC.rearrange("p (c j) -> p c j", j=L),
            in_=notselC.rearrange("p (c j) -> p c j", j=L),
            compare_op=ALU.not_equal,
            fill=0.0,
            base=0,
            pattern=[[-1, CDD], [0, L]],
            channel_multiplier=1,
        )
        # Junk suppression for the pair-quadrant block scores: block c's
        # useful key rows are 64*(c%2); the other 64 rows are junk. A rank-2
        # fp8 DoubleRow matmul adds -96 to the junk quadrants IN PSUM, so
        # exp turns them into e^-12 ~ 0 and the block PV matmuls can run
        # full-height over a whole pair. jrow plane0 selects rows 0:64,
        # plane1 rows 64:128; jneg has -96 on the matching junk columns.

        # ---- x stream (b=0) on the SP queue in ktpair-interleaved chunks
        # so the Q projection starts after ~1us and proceeds as pairs land.
        x_cur = {
            "x8": xtp.tile([P, KT, S], FP8, tag="x8", name="x8t"),
            "dxt8": xtp.tile([P, KT, T], FP8, tag="dxt8", name="dxt8t"),
            "xt64": xtp.tile([P, KT, T], FP8, tag="xt64", name="xt64t"),
        }
        x8r = x8[0].rearrange("(kt p) s -> p kt s", p=P)
        for kts in (slice(0, 2), slice(2, 4), slice(4, KT)):
            nc.sync.dma_start(out=x_cur["x8"][:, kts, :], in_=x8r[:, kts, :])
        b_col = {}
        for name, bap in (("q", bq), ("k", bk)):
            bc = cpool.tile([P, KT], FP32, tag=f"bc{name}", name=f"bcol{name}")
            nc.sync.dma_start(out=bc[:], in_=bap.rearrange("(t p) -> p t", p=P))
            b_col[name] = bc
        bv_row = cpool.tile([1, H], BF16)
        nc.sync.dma_start(out=bv_row[:], in_=bv16[None, :])
        nc.sync.dma_start(
            out=x_cur["dxt8"][:], in_=dxt8[0].rearrange("(kt p) s -> p kt s", p=P)
        )
        nc.sync.dma_start(
            out=x_cur["xt64"][:], in_=xt64[0].rearrange("(kt p) s -> p kt s", p=P)
        )
        bvb = cpool.tile([P, H], FP32)  # materialized [128, H] V bias

        # ---- per-batch x tiles (both batches up front; b>0 DMAs are
        # emitted mid-batch-0 as prefetch) ----
        xcs = [x_cur]
        for b in range(1, BL):
            xcs.append({
                "x8": xtp.tile([P, KT, S], FP8, tag="x8", name="x8t"),
                "dxt8": xtp.tile([P, KT, T], FP8, tag="dxt8", name="dxt8t"),
                "xt64": xtp.tile([P, KT, T], FP8, tag="xt64", name="xt64t"),
            })

        def batch_units(b):
            """Return (pre, attn, tail) emission-thunk lists. Ordering is
            tuned for the 2-slot 3-bank PSUM rotation: consecutive psbig
            allocations alternate drain engines (ACT exps / DVE copies)
            with enough PE work in between to cover each drain."""
            xc = xcs[b]
            st = {}
            st["q8"] = [qkv.tile([P, 2, NQ], FP8, tag=f"q8{m}", name=f"q8{m}") for m in range(3)]
            st["k8"] = [qkv.tile([P, 2, S], FP8, tag=f"k8{m}", name=f"k8{m}") for m in range(3)]
            st["vext"] = [qkv.tile([P, NH * VW], BF16, tag=f"v{mt}", name=f"v{mt}") for mt in range(KT)]
            st["vsumsE"] = smp.tile([CDD, NH * VW], BF16, tag="vsums", name="vsumsE")
            st["se"] = [None] * NH
            st["ostage"] = [
                osp.tile([P, H], BF16, tag=f"os{j}", name=f"os{j}")
                for j in range(5)
            ]

            def proj_qk(name, t):
                def th():
                    dst, ntot = (st["q8"], NQ) if name == "q" else (st["k8"], S)
                    m, half = divmod(t, 2)
                    ts_ = slice(t * P, (t + 1) * P)
                    ps = psbig()
                    for n0 in range(0, ntot, 512):
                        nlen = min(512, ntot - n0)
                        for i in range(KTP):
                            nc.tensor.matmul(
                                ps[:, n0 : n0 + nlen],
                                lhsT=w_sb[name][:, 2 * i : 2 * i + 2, ts_],
                                rhs=xc["x8"][:, 2 * i : 2 * i + 2, n0 : n0 + nlen],
                                start=(i == 0),
                                stop=(i == KTP - 1),
                                perf_mode=DR,
                            )
                    # drain-engine split tuned so ACT (exps, some drains) and
                    # DVE (everything else) finish together
                    on_act = name == "k" and t < 2
                    if on_act:
                        nc.scalar.activation(
                            dst[m][:, half, :],
                            ps[:, 0:ntot],
                            AF.Identity,
                            bias=b_col[name][:, t : t + 1],
                        )
                    else:
                        nc.vector.tensor_scalar_add(
                            dst[m][:, half, :],
                            ps[:, 0:ntot],
                            b_col[name][:, t : t + 1],
                        )
                return th

            def emit_scores(h):
                # terms + blocks c8/c9 in a 2-bank psT tile; blocks c0..c7
                # in a 1-bank psU tile. Pair-quadrant block matmuls (half
                # the rows junk) keep every dual-fp8 dst at partition 0.
                # Two exps per head, each draining its own tile.
                def th():
                    m, hl = divmod(h, HGS)
                    rows = slice(32 * hl, 32 * hl + 32)
                    tp = (32 * hl, 0)
                    q8, k8 = st["q8"], st["k8"]

                    def pairmm(ps, col, j, start, stop):
                        # both blocks of pair j in one matmul: same lhsT (the
                        # pair's 128 key rows), contiguous 128 q columns
                        js = slice(2 * j * L, (2 * j + 2) * L)
                        nc.tensor.matmul(
                            ps[:, col : col + P],
                            lhsT=k8[m][rows, :, js],
                            rhs=q8[m][rows, :, js],
                            start=start,
                            stop=stop,
                            perf_mode=DR,
                            skip_group_check=True,
                            tile_position=tp,
                        )

                    se = sep.tile([P, SW], BF16, tag=f"se{h}", name=f"se{h}")
                    pst = psbig()
                    nc.tensor.matmul(
                        pst[:, 0:512],
                        lhsT=k8[m][rows, :, NQ:S],
                        rhs=q8[m][rows, :, 0:512],
                        start=True,
                        stop=True,
                        perf_mode=DR,
                        tile_position=tp,
                    )
                    nc.tensor.matmul(
                        pst[:, 512:640],
                        lhsT=k8[m][rows, :, NQ:S],
                        rhs=q8[m][rows, :, 512:640],
                        start=True,
                        stop=False,
                        perf_mode=DR,
                        skip_group_check=True,
                        tile_position=tp,
                    )
                    pairmm(pst, 640, 4, False, True)
                    nc.scalar.activation(
                        se[:, 0:768], pst[:, 0:768], AF.Exp, scale=0.125
                    )
                    psb = psu()
                    for j in range(4):
                        pairmm(psb, j * P, j, j == 0, j == 3)
                    nc.scalar.activation(
                        se[:, 768:SW], psb[:, 0:512], AF.Exp, scale=0.125
                    )
                    # zero the junk quadrants so block PV matmuls can run
                    # full-height: in se's block region [c8 c9 c0..c7], even
                    # region-index blocks have junk rows 64:128, odd 0:64
                    sbv = se[:, 640:SW].rearrange("p (c j) -> p c j", j=L)
                    nc.gpsimd.memset(sbv[64:128, 0::2, :], 0.0)
                    nc.gpsimd.memset(sbv[0:64, 1::2, :], 0.0)
                    st["se"][h] = se
                return th

            def v_copy(mt, ps):
                vv = st["vext"][mt].rearrange("p (h c) -> p h c", c=VW)
                nc.vector.tensor_tensor(
                    out=vv[:, :, 0:DH],
                    in0=ps[:, 0:H].rearrange("p (h c) -> p h c", c=DH),
                    in1=bvb[:].rearrange("p (h c) -> p h c", c=DH),
                    op=ALU.add,
                )
                nc.gpsimd.memset(vv[:, :, DH : DH + 1], 1.0)

            def make_bvb():
                # materialized V bias [128, H] fp32 (b==0 only)
                ps = psbig()
                for n0, nlen in ((0, 512), (512, 256)):
                    nc.tensor.matmul(
                        ps[:, n0 : n0 + nlen],
                        lhsT=onesrow[:],
                        rhs=bv_row[0:1, n0 : n0 + nlen],
                        start=True,
                        stop=True,
                    )
                nc.vector.tensor_copy(bvb[:], ps[:, 0:H])

            def v_mt5():
                # term rows feed the output directly, so they get a
                # second-order-accurate residual-fp8 product:
                #   X16'W16 ~ X8'W8 + X8'dW8 + dX8'W8   (X = X8 + dX, ...)
                # The bias comes in via a rank-1 PSUM opener so the two
                # drains (vterm on ACT, vext5 on DVE) run in parallel.
                ps = psbig()
                for n0, nlen in ((0, 512), (512, 256)):
                    nc.tensor.matmul(
                        ps[:, n0 : n0 + nlen],
                        lhsT=onesrow[:],
                        rhs=bv_row[0:1, n0 : n0 + nlen],
                        start=True,
                        stop=False,
                    )
                    for lhs, rhs in (
                        (xc["x8"], w_sb["v"]),
                        (xc["xt64"], dwv8_sb),
                        (xc["dxt8"], w_sb["v"]),
                    ):
                        for i in range(KTP):
                            lt = (
                                lhs[:, 2 * i : 2 * i + 2, NQ:S]
                                if lhs is xc["x8"]
                                else lhs[:, 2 * i : 2 * i + 2, :]
                            )
                            nc.tensor.matmul(
                                ps[:, n0 : n0 + nlen],
                                lhsT=lt,
                                rhs=rhs[:, 2 * i : 2 * i + 2, n0 : n0 + nlen],
                                start=False,
                                stop=(rhs is None),
                                perf_mode=DR,
                            )
                    # close the group with a rank-1 +0
                    nc.tensor.matmul(
                        ps[:, n0 : n0 + 1],
                        lhsT=onesrow[:],
                        rhs=zrow[:],
                        start=False,
                        stop=True,
                    )
                vv = st["vext"][5].rearrange("p (h c) -> p h c", c=VW)
                nc.vector.tensor_copy(
                    vv[:, :, 0:DH],
                    ps[:, 0:H].rearrange("p (h c) -> p h c", c=DH),
                )
                nc.gpsimd.memset(vv[:, :, DH : DH + 1], 1.0)
                # term rows pass through V - DMA out early, strided straight
                # from vext[5] (skips the separate vterm staging copy)
                nc.sync.dma_start(
                    out=out[b][NQ:S, :].rearrange("p (h c) -> p h c", c=DH),
                    in_=vv[:, :, 0:DH],
                )

            def v_mt(mt):
                # candidate rows: fp8 DoubleRow
                def th():
                    ms = slice(mt * P, (mt + 1) * P)
                    ps = psbig()
                    for n0, nlen in ((0, 512), (512, 256)):
                        for i in range(KTP):
                            nc.tensor.matmul(
                                ps[:, n0 : n0 + nlen],
                                lhsT=xc["x8"][:, 2 * i : 2 * i + 2, ms],
                                rhs=w_sb["v"][:, 2 * i : 2 * i + 2, n0 : n0 + nlen],
                                start=(i == 0),
                                stop=(i == KTP - 1),
                                perf_mode=DR,
                            )
                    v_copy(mt, ps)
                return th

            def vsum():
                # per-block value sums (Vsum_c = Xsum_c @ Wv + 64*bv, 65th
                # col = 64.0) are precomputed on the host and DMA'd in
                nc.sync.dma_start(out=st["vsumsE"][:], in_=vse[b])

            def prefetch():
                if b + 1 < BL:
                    xn = xcs[b + 1]
                    nc.gpsimd.dma_start(
                        out=xn["x8"][:],
                        in_=x8[b + 1].rearrange("(kt p) s -> p kt s", p=P),
                    )
                    nc.gpsimd.dma_start(
                        out=xn["dxt8"][:],
                        in_=dxt8[b + 1].rearrange("(kt p) s -> p kt s", p=P),
                    )
                    nc.gpsimd.dma_start(
                        out=xn["xt64"][:],
                        in_=xt64[b + 1].rearrange("(kt p) s -> p kt s", p=P),
                    )

            def emit_pv_j(hg, j):
                def th():
                    vext, vsumsE, se_h = st["vext"], st["vsumsE"], st["se"]
                    hgs_v = slice(hg * HGS * VW, (hg + 1) * HGS * VW)
                    psc = psu()
                    # head 0's full-height terms matmul opens the bank's one
                    # accumulation group; everything else accumulates.
                    for hl in range(HGS):
                        h = hg * HGS + hl
                        vs = slice(h * VW, (h + 1) * VW)
                        nc.tensor.matmul(
                            psc[:, hl * VW : (hl + 1) * VW],
                            lhsT=se_h[h][:, _tcol(j) : _tcol(j) + P],
                            rhs=vext[5][:, vs],
                            start=(hl == 0),
                            stop=False,
                        )
                    # both 64-row halves in one full-height matmul each: the
                    # pair's q columns are contiguous in notselC and se, and
                    # the junk quadrants of se are zeroed
                    nc.tensor.matmul(
                        psc[:, 0 : HGS * VW],
                        lhsT=notselC[:, 2 * j * L : (2 * j + 2) * L],
                        rhs=vsumsE[:, hgs_v],
                        start=False,
                        stop=False,
                    )
                    for hl in range(HGS):
                        h = hg * HGS + hl
                        vs = slice(h * VW, (h + 1) * VW)
                        nc.tensor.matmul(
                            psc[:, hl * VW : hl * VW + VW],
                            lhsT=se_h[h][:, _pcol(j) : _pcol(j) + P],
                            rhs=vext[j][:, vs],
                            start=False,
                            stop=(hl == HGS - 1),
                        )
                    zr = smp.tile([P, HGS], FP32, tag="zr", bufs=4, name="zr")
                    pscv = psc[:, 0 : HGS * VW].rearrange("p (h c) -> p h c", c=VW)
                    nc.vector.reciprocal(
                        zr[:].rearrange("p (h o) -> p h o", o=1),
                        pscv[:, :, DH : DH + 1],
                    )
                    ov = st["ostage"][j].rearrange("p (h c) -> p h c", c=DH)
                    bin0, bin1 = bass.broadcast_tensor_aps(
                        pscv[:, :, 0:DH],
                        zr[:].rearrange("p (h o) -> p h o", o=1),
                    )
                    nc.vector.tensor_tensor(
                        out=ov[:, hg * HGS : (hg + 1) * HGS, :],
                        in0=bin0,
                        in1=bin1,
                        op=ALU.mult,
                    )
                    # split the output DMA: first 2 groups' columns go out
                    # after PV(1,j); the last group's after PV(2,j)
                    if hg == 1:
                        nc.sync.dma_start(
                            out=out[b][j * P : (j + 1) * P, 0 : 2 * HGS * DH],
                            in_=st["ostage"][j][:, 0 : 2 * HGS * DH],
                        )
                    elif hg == 2:
                        nc.sync.dma_start(
                            out=out[b][j * P : (j + 1) * P, 2 * HGS * DH : H],
                            in_=st["ostage"][j][:, 2 * HGS * DH : H],
                        )
                return th

            def proj_q_triple(ts3):
                # pair-major over three Q tiles: all three advance as each
                # x8/wq8 ktpair chunk lands, so the DMA-gated startup window
                # keeps the PE fed instead of serializing per tile
                def th():
                    pss = [psbig() for _ in ts3]
                    for i in range(KTP):
                        for idx, t in enumerate(ts3):
                            ts_ = slice(t * P, (t + 1) * P)
                            for n0 in range(0, NQ, 512):
                                nlen = min(512, NQ - n0)
                                nc.tensor.matmul(
                                    pss[idx][:, n0 : n0 + nlen],
                                    lhsT=w_sb["q"][:, 2 * i : 2 * i + 2, ts_],
                                    rhs=xc["x8"][:, 2 * i : 2 * i + 2, n0 : n0 + nlen],
                                    start=(i == 0),
                                    stop=(i == KTP - 1),
                                    perf_mode=DR,
                                )
                    for idx, t in enumerate(ts3):
                        m, half = divmod(t, 2)
                        nc.vector.tensor_scalar_add(
                            st["q8"][m][:, half, :],
                            pss[idx][:, 0:NQ],
                            b_col["q"][:, t : t + 1],
                        )
                return th

            Q = [proj_qk("q", t) for t in range(KT)]
            K = [proj_qk("k", t) for t in range(KT)]
            SC = [emit_scores(h) for h in range(NH)]
            V = [v_mt(mt) for mt in range(5)]
            PV = {(hg, j): emit_pv_j(hg, j) for hg in range(NHG) for j in range(5)}
            pre = [
                Q[0], K[0], Q[1], K[1],
                SC[0], Q[2], K[2], SC[1], Q[3], K[3],
                SC[2], Q[4], K[4], SC[3], Q[5], K[5],
                prefetch,
            ]
            if b == 0:
                pre.append(make_bvb)
            pre += [
                vsum, SC[4], v_mt5, SC[5], V[0], SC[6], V[1], SC[7],
            ]
            # V[2..4] arrive just-in-time inside the attention weave
            attn = [
                SC[8], PV[0, 0], V[2], PV[0, 1],
                SC[9], PV[0, 2], V[3], PV[0, 3],
                SC[10], V[4], PV[0, 4], PV[1, 0],
                SC[11], PV[1, 1], PV[1, 2], PV[1, 3], PV[1, 4],
            ]
            tail = [PV[2, j] for j in range(5)]
            return pre, attn, tail

        u0 = batch_units(0)
        u1 = batch_units(1)
        for th in u0[0]:
            th()
        # weave batch 1's projections/scores into batch 0's attention and
        # final PV group: batch 0's PV chunks are pure PE work that covers
        # batch 1's ACT/DVE copy and exp drains (and vice versa)
        rest0 = u0[1] + u0[2]
        pre1 = u1[0]
        k = 0
        for i, th in enumerate(rest0):
            th()
            if i >= 8:  # let batch 0's attention pipeline fill first
                for _ in range(2):
                    if k < len(pre1):
                        pre1[k]()
                        k += 1
        while k < len(pre1):
            pre1[k]()
            k += 1
        for th in u1[1]:
            th()
        for th in u1[2]:
            th()


_CACHE = {}


def _get_program():
    if "nc" not in _CACHE:
        _CACHE["nc"] = _build_program()
    return _CACHE["nc"]


NPF8 = ml_dtypes.float8_e4m3
NPBF = ml_dtypes.bfloat16


def _make_in_maps(inputs):
    hs = np.asarray(inputs["hidden_states"], np.float32)
    hst = np.ascontiguousarray(hs.transpose(0, 2, 1))  # [B, H, S]
    x8 = hst.astype(NPF8)
    dxt8 = (
        hst[:, :, NQ:] - x8[:, :, NQ:].astype(np.float32)
    ).astype(NPF8)
    xt64 = (hst[:, :, NQ:] / 64.0).astype(NPF8)
    xsum = hst[:, :, :NQ].reshape(B, H, CDD, L).sum(axis=3)  # [B, H, CDD]

    # Q/K projection out-column permutation: out-tile t=2m+half holds
    # (head 4m + r//32, dh = 32*half + r%32) at partition r.
    r = np.arange(P)
    perm = np.empty(H, np.int64)
    for t in range(KT):
        m, half = divmod(t, 2)
        perm[t * P + r] = (HGS * m + r // 32) * DH + 32 * half + (r % 32)

    wq = np.asarray(inputs["Wq"], np.float32).T
    wk = np.asarray(inputs["Wk"], np.float32).T
    wv = np.asarray(inputs["Wv"], np.float32).T
    bqp = np.asarray(inputs["bq"], np.float32)[perm]
    bkp = np.asarray(inputs["bk"], np.float32)[perm]
    bv = np.asarray(inputs["bv"], np.float32)

    # host-precomputed per-block value sums (full fp32 precision):
    # Vsum[b, c, :] = Xsum_c @ Wv^T + 64*bv, extended per head with a 65th
    # column of 64.0 (so the notselC correction also contributes 9*64 to Z)
    vsum = np.einsum("bic,io->bco", xsum, wv) + L * bv  # [B, CDD, H]
    vse = np.full((B, CDD, NH, VW), float(L), np.float32)
    vse[:, :, :, :DH] = vsum.reshape(B, CDD, NH, DH)
    vse = vse.reshape(B, CDD, NH * VW).astype(NPBF)
    in_common = {
        "wq8": np.ascontiguousarray(wq[:, perm]).astype(NPF8),
        "wk8": np.ascontiguousarray(wk[:, perm]).astype(NPF8),
        "wv8": np.ascontiguousarray(wv).astype(NPF8),
        "dwv8": np.ascontiguousarray(
            64.0 * (wv - np.ascontiguousarray(wv).astype(NPF8).astype(np.float32))
        ).astype(NPF8),
        "bq": np.ascontiguousarray(bqp),
        "bk": np.ascontiguousarray(bkp),
        "bv16": bv.astype(NPBF),
    }
    return [
        {
            "x8": x8[i * BL : (i + 1) * BL],
            "dxt8": dxt8[i * BL : (i + 1) * BL],
            "xt64": xt64[i * BL : (i + 1) * BL],
            "vse": vse[i * BL : (i + 1) * BL],
            **in_common,
        }
        for i in range(NCORES)
    ]


def kernel(**inputs) -> np.ndarray:
    in_maps = _make_in_maps(inputs)
    nc = _get_program()
    res = run_bass_kernel_spmd(nc, in_maps, list(range(NCORES)))
    return np.concatenate(
        [res.results[i]["out"].astype(np.float32) for i in range(NCORES)], axis=0
    )

